# revision 1
# baseline (speedup 1.0000x reference)
"""Causal attention pixel block kernel for Trainium2 (8 NeuronCores).

Problem: 3 directional stacks x batch 1 x 8 heads of causal attention over
S=2048 flattened spatial positions, head dim 8 (64 channels total), fp32.

Sharding: the 3*1*8 = 24 (stack, head) units are data/head-parallel; each of
the 8 cores processes 3 units end-to-end (full 2048x2048 logits for its
units). The causal mask is the deterministic lower-triangular mask from the
reference; it is implemented on-chip (block skipping + a triangular mask on
diagonal blocks), so the attn_mask input never needs to reach the device.

Per-unit device pipeline (all fp32):
  scoresT[j, i] = sum_c k[c, j] q[c, i]      (PE, K=8 matmuls, j-tiles of 128)
  wT = exp(scoresT / sqrt(8))                (ScalarE, PSUM -> SBUF)
  diagonal blocks: wT *= upper-tri mask      (VectorE)
  outT[c, i] = sum_j vaug[j, c] wT[j, i]     (PE, accumulated over j-tiles)
    where vaug has a ones-column: row c=8 of outT is the softmax denominator
  out = outT[0:8] * recip(outT[8])           (VectorE + GpSimd broadcast)

The i-axis is processed in halves of 1024 so PSUM holds two double-buffered
[128, 1024] score tiles plus two [9, 1024] output accumulators (8 banks).
"""

import math

import numpy as np

import concourse.bass as bass
import concourse.tile as tile
from concourse import bacc, mybir
from concourse.bass_utils import run_bass_kernel_spmd
from concourse.masks import make_upper_triangular

N_CORES = 8
STACK, B, C, D, H, W = 3, 1, 64, 8, 16, 16
S = D * H * W                  # 2048 attention positions
NH = 8                         # num heads
CK = C // NH                   # head dim = 8
UNITS = STACK * B * NH         # 24
UPC = UNITS // N_CORES         # 3 units per core
NJT = S // 128                 # 16 j-tiles per unit
AVW = 40                       # AV lhsT width: ones col at 0 (rowsum lands at
                               # PSUM partition 0 where the fast-reciprocal
                               # custom op can read it), v in cols 32..39
                               # (partition 32 is a legal engine base)
HALF = S // 2                  # i-axis processed in halves of 1024
SCALE = CK ** -0.5

F32 = mybir.dt.float32
# fp32 matmuls stream at 4 cycles/row on the PE; float32r (same bits) streams
# at 1 cycle/row for moving dims >= 256.
F32R = mybir.dt.float32r

# tuning knobs (module-level so sweep scripts can override before build)
QK_BUFS = 2      # PSUM double-buffering for score tiles ([128, HALF] = 2 banks)
AV_BUFS = 2      # PSUM buffering for the [40, 1024] output accumulators (2 banks)
W_BUFS = 4       # SBUF buffering for exp'd score tiles
O_BUFS = 6       # SBUF buffering for the normalize/output tiles
DIAG_LAST = False # emit the mask-dependent diagonal AV chunk after the others
PE_WARMUP = 4      # dummy matmuls to release the HAM clock throttle early
FINE_TAIL = True   # 512-wide normalize chunks on the very last half only
BCAST_DMA = False  # broadcast recip row via DRAM-bounce DMA vs gpsimd
NORM_CHUNK = 1024  # width of the normalize/output chains (512 or 1024)
ABLATE = ""        # timing ablations: "qk" | "exp" | "av" | "" (full)
REPS = 1         # repeat the whole compute (for calibration benchmarks only)


def _emit(tc: tile.TileContext, q_d, k_d, v_d, o_d):
    nc = tc.nc
    Exp = mybir.ActivationFunctionType.Exp

    with (
        tc.tile_pool(name="singles", bufs=1) as singles,
        tc.tile_pool(name="w", bufs=W_BUFS) as wpool,
        tc.tile_pool(name="out", bufs=O_BUFS) as opool,
        tc.tile_pool(name="qk", bufs=QK_BUFS, space="PSUM") as qkpool,
        tc.tile_pool(name="av", bufs=AV_BUFS, space="PSUM") as avpool,
        tc.tile_pool(name="dram", bufs=O_BUFS, space="DRAM") as dpool,
    ):
        # trigger the ACT exp table load immediately so it overlaps the
        # input DMAs instead of stalling the first real exp (~2.7us)
        warm = singles.tile([1, 1], F32)
        nc.vector.memset(warm, 0.0)
        nc.scalar.activation(warm, warm, Exp, scale=1.0)

        q_sb = singles.tile([CK, UPC, S], F32R)
        k_sb = singles.tile([CK, UPC, S], F32R)
        v_sb = singles.tile([128, UPC, NJT, AVW], F32R)
        # priority slices: just what the first QK row needs (k j-tile 0 and
        # the first half of q for unit 0), so compute starts ~2us earlier
        nc.sync.dma_start(out=k_sb[:, 0, 0:128], in_=k_d.ap()[:, 0, 0:128])
        nc.sync.dma_start(out=q_sb[:, 0, 0:HALF], in_=q_d.ap()[:, 0, 0:HALF])
        # bulk loads (exclude the priority slices to avoid a rewrite stall)
        nc.sync.dma_start(out=k_sb[:, 0, 128:S], in_=k_d.ap()[:, 0, 128:S])
        nc.sync.dma_start(out=q_sb[:, 0, HALF:S], in_=q_d.ap()[:, 0, HALF:S])
        nc.sync.dma_start(out=v_sb[:, 0, :, :], in_=v_d.ap()[:, 0, :, :])
        for u in range(1, UPC):
            nc.sync.dma_start(out=k_sb[:, u, :], in_=k_d.ap()[:, u, :])
            nc.sync.dma_start(out=q_sb[:, u, :], in_=q_d.ap()[:, u, :])
            nc.sync.dma_start(out=v_sb[:, u, :, :], in_=v_d.ap()[:, u, :, :])

        # trimask[p, f] = 1.0 if f >= p else 0.0 (keep j <= i on diag blocks)
        trimask = singles.tile([128, 128], F32)
        make_upper_triangular(nc, trimask[:], val=1.0, diag=True)

        if PE_WARMUP:
            # dummy matmuls during the input DMA wait: ~3.4us of PE activity
            # releases the HAM clock throttle (1.2 -> 2.4 GHz) before the
            # first real QK matmul
            wsrc = singles.tile([CK, 512], F32R)
            nc.vector.memset(wsrc.bitcast(F32), 0.0)
            wp = qkpool.tile([128, HALF], F32, tag='qk')
            for _ in range(PE_WARMUP):
                nc.tensor.matmul(
                    wp[:, 0:512],
                    lhsT=wsrc[:, 0:128],
                    rhs=wsrc,
                    start=True,
                    stop=True,
                )

        for _rep in range(REPS):
            for u in range(UPC):
                for hf in range(2):
                    base = hf * HALF           # absolute i offset of this half
                    jt_end = (hf + 1) * (HALF // 128)
                    av = avpool.tile([AVW, HALF], F32)

                    def emit_qk(jt):
                        s0 = max(jt * 128, base) - base
                        qk = qkpool.tile([128, HALF], F32)
                        for c0 in range(0, HALF, 512):
                            lo = max(c0, s0)
                            if lo >= c0 + 512:
                                continue
                            nc.tensor.matmul(
                                qk[:, lo:c0 + 512],
                                lhsT=k_sb[:, u, jt * 128:(jt + 1) * 128],
                                rhs=q_sb[:, u, base + lo:base + c0 + 512],
                                start=True,
                                stop=True,
                            )
                        return qk

                    qk = emit_qk(0)
                    for jt in range(jt_end):
                        s0 = max(jt * 128, base) - base
                        wt = wpool.tile([128, HALF], F32R)
                        if ABLATE != "qk":
                            nc.scalar.activation(
                                wt[:, s0:HALF], qk[:, s0:HALF], Exp, scale=SCALE
                            )
                        # issue the next row's QK before this row's AV so the
                        # PE keeps ScalarE fed instead of running in lockstep
                        if jt + 1 < jt_end:
                            qk = emit_qk(jt + 1)
                        if ABLATE in ("qk", "exp"):
                            continue
                        if jt * 128 >= base:
                            # diagonal block: zero out j > i entries
                            nc.vector.tensor_mul(
                                wt[:, s0:s0 + 128], wt[:, s0:s0 + 128], trimask
                            )
                        for c0 in range(0, HALF, 512):
                            lo = max(c0, s0)
                            if lo >= c0 + 512:
                                continue
                            last_jt = min(jt_end - 1, (base + c0 + 512) // 128 - 1)
                            nc.tensor.matmul(
                                av[:, lo:c0 + 512],
                                lhsT=v_sb[:, u, jt, :],
                                rhs=wt[:, lo:c0 + 512],
                                start=(jt == 0),
                                stop=(jt == last_jt),
                                skip_group_check=True,
                            )
                        # normalize any chunk-wide output chunk that just
                        # finished accumulating (low chunks finish early):
                        # out = outT[32:40] * recip(rowsum row 0)
                        if ABLATE:
                            continue
                        nchunk = NORM_CHUNK
                        if FINE_TAIL and u == UPC - 1 and hf == 1:
                            nchunk = 512
                        for c in range(HALF // nchunk):
                            cl, ch = nchunk * c, nchunk * (c + 1)
                            if min(jt_end - 1, (base + ch) // 128 - 1) != jt:
                                continue
                            sl = slice(cl, ch)
                            r = opool.tile([1, nchunk], F32)
                            nc.vector.reciprocal_approx_fast(
                                out=r, in_=av[0:1, sl]
                            )
                            rb = opool.tile([CK, nchunk], F32)
                            if BCAST_DMA:
                                # bounce through DRAM: a partition-stride-0
                                # read (broadcast) is only legal from DRAM,
                                # and DMA costs no compute-engine time
                                rd = dpool.tile([1, NORM_CHUNK], F32)
                                nc.sync.dma_start(out=rd, in_=r)
                                rd_b = bass.AP(
                                    tensor=rd.tensor, offset=rd.offset,
                                    ap=[[0, CK]] + list(rd.ap[1:]),
                                )
                                nc.sync.dma_start(out=rb, in_=rd_b)
                            else:
                                nc.gpsimd.partition_broadcast(rb, r, channels=CK)
                            osb = opool.tile([CK, nchunk], F32)
                            nc.vector.tensor_mul(osb, av[32:32 + CK, sl], rb)
                            nc.sync.dma_start(
                                out=o_d.ap()[u, :, base + cl:base + ch],
                                in_=osb,
                            )


_PROGRAM = None


def _get_program():
    global _PROGRAM
    if _PROGRAM is None:
        nc = bacc.Bacc(
            "TRN2",
            target_bir_lowering=False,
            debug=False,
            num_devices=N_CORES,
        )
        q_d = nc.declare_dram_parameter("q", [CK, UPC, S], F32R, isOutput=False)
        k_d = nc.declare_dram_parameter("k", [CK, UPC, S], F32R, isOutput=False)
        v_d = nc.declare_dram_parameter(
            "vaug", [128, UPC, NJT, AVW], F32R, isOutput=False
        )
        o_d = nc.declare_dram_parameter("o", [UPC, CK, S], F32, isOutput=True)
        with tile.TileContext(nc) as tc:
            _emit(tc, q_d, k_d, v_d, o_d)
        if not nc.is_finalized():
            nc.finalize()
        _PROGRAM = nc
    return _PROGRAM


# test.py can flip this on to capture an NTFF trace / exec time.
TRACE = False
LAST_RESULTS = None


def kernel(keys, queries, values, attn_mask, num_heads):
    global LAST_RESULTS
    nh = int(num_heads)
    assert nh == NH, f"compiled for num_heads={NH}, got {nh}"
    assert keys.shape == (STACK, B, C, D, H, W)

    # (stack*b, head, ck, seq)
    q = np.ascontiguousarray(queries, np.float32).reshape(STACK * B, NH, CK, S)
    k = np.ascontiguousarray(keys, np.float32).reshape(STACK * B, NH, CK, S)
    v = np.ascontiguousarray(values, np.float32).reshape(STACK * B, NH, CK, S)

    in_maps = []
    for core in range(N_CORES):
        units = range(core * UPC, (core + 1) * UPC)
        qs = np.stack([q[u // NH, u % NH] for u in units], 1)  # [CK, UPC, S]
        ks = np.stack([k[u // NH, u % NH] for u in units], 1)
        vt = np.stack([v[u // NH, u % NH] for u in units], 0)  # [UPC, CK, S]
        vaug = np.zeros((128, UPC, NJT, AVW), np.float32)
        vaug[:, :, :, 32:32 + CK] = vt.reshape(UPC, CK, NJT, 128).transpose(3, 0, 2, 1)
        vaug[:, :, :, 0] = 1.0
        in_maps.append(
            {
                "q": np.ascontiguousarray(qs),
                "k": np.ascontiguousarray(ks),
                "vaug": vaug,
            }
        )

    nc = _get_program()
    kwargs = {}
    if TRACE:
        kwargs = dict(trace=True, trace_cores=[0])
    LAST_RESULTS = run_bass_kernel_spmd(
        nc, in_maps, core_ids=list(range(N_CORES)), **kwargs
    )

    out = np.empty((STACK * B, NH, CK, S), np.float32)
    for core in range(N_CORES):
        o = LAST_RESULTS.results[core]["o"]  # [UPC, CK, S]
        for j, u in enumerate(range(core * UPC, (core + 1) * UPC)):
            out[u // NH, u % NH] = o[j]
    return out.reshape(STACK, B, C, D, H, W)



# revision 7
# speedup vs baseline: 1.1245x; 1.1245x over previous
"""Causal attention pixel block kernel for Trainium2 (8 NeuronCores).

Problem: 3 directional stacks x batch 1 x 8 heads of causal attention over
S=2048 flattened spatial positions, head dim 8 (64 channels total), fp32.

Sharding: the 3*1*8 = 24 (stack, head) units are data/head-parallel; each of
the 8 cores processes 3 units end-to-end (full 2048x2048 logits for its
units). The causal mask is the deterministic lower-triangular mask from the
reference; it is implemented on-chip (block skipping + a triangular mask on
diagonal blocks), so the attn_mask input never needs to reach the device.

Dataflow per unit (all fp32; ACT-engine/exp-bound by design):
  scoresT[j, i] = sum_c k[c, j] q[c, i]      (PE, K=8 matmuls, j-tiles of 128)
  wT = exp(scoresT / sqrt(8))                (ScalarE; one activation per
                                              multi-row PSUM tile)
  diagonal blocks: wT *= upper-tri mask      (VectorE, [128,128])
  av[i, 0:9] += wtT[j-block, i-block].T @ vaug[j-block, 0:9]
                                             (PE; vaug col 0 is all-ones so
                                              av[:, 0] is the softmax denom;
                                              cols 1..8 are the v values)
  out[i, c] = av[i, 1+c] * recip(av[i, 0])   (VectorE only: batched
                                              reciprocal + stride-0 broadcast
                                              multiply; no gpsimd)

The causal triangle (row j-tile jt covers i in [128*jt, 2048)) is decomposed
into width-{1024,512,384,256,128} segments packed into two PSUM tile shapes:
"B" [128,2,1024] (4 banks) and "S" [128,<=4KB] (2 banks), processed strictly
alternating B,S,B,S,... so the ScalarE exp stream never waits for QK. One exp
instruction covers a whole tile (12 activations per unit instead of 24+).
PSUM budget: B(4) + S(2) + av ring [128,2,16,9] (1) = 7 of 8 banks.
"""

import math

import numpy as np

import concourse.bass as bass
import concourse.tile as tile
from concourse import bacc, mybir
from concourse.bass_utils import run_bass_kernel_spmd
from concourse.masks import make_upper_triangular

N_CORES = 8
STACK, B, C, D, H, W = 3, 1, 64, 8, 16, 16
S = D * H * W                  # 2048 attention positions
NH = 8                         # num heads
CK = C // NH                   # head dim = 8
UNITS = STACK * B * NH         # 24
UPC = UNITS // N_CORES         # 3 units per core
NJT = S // 128                 # 16 j-tiles (and i-tiles) per unit
AVW = 1 + CK                   # av columns: rowsum at 0, v at 1..8
SCALE = CK ** -0.5

F32 = mybir.dt.float32
# fp32 matmuls stream at 4 cycles/row on the PE; float32r (same bits) streams
# at 1 cycle/row for moving dims >= 256.
F32R = mybir.dt.float32r

# tuning knobs
PE_WARMUP = 4      # dummy matmuls to release the HAM clock throttle early
WT_BUFS = 2        # SBUF buffering for exp'd score tiles (per tag)
O_BUFS = 2         # SBUF buffering for normalize/output tiles


def _unit_tiles():
    """Static per-unit schedule: 12 tiles, each a list of row-segments
    (row, jt, i0, w, col0) packed in a PSUM tile of geometry (nrows, rowlen);
    geometry rowlen is the padded per-row length (bank aligned)."""
    seg = lambda r, jt, i0, w, c0: dict(r=r, jt=jt, i0=i0, w=w, c0=c0)
    tiles = []

    def add(kind, nrows, rowlen, expw, segs):
        tiles.append(dict(kind=kind, nrows=nrows, rowlen=rowlen, expw=expw,
                          segs=segs))

    # B tiles: [128, 2, 1024] (4 banks). S tiles: <= 4KB/partition (2 banks).
    add('B', 2, 1024, 1024, [seg(0, 0, 0, 1024, 0), seg(1, 0, 1024, 1024, 0)])
    add('S', 2, 512, 512, [seg(0, 9, 1152, 512, 0), seg(1, 10, 1280, 512, 0)])
    add('B', 2, 1024, 1024, [seg(0, 1, 128, 1024, 0), seg(1, 2, 256, 1024, 0)])
    add('S', 2, 512, 512, [seg(0, 11, 1408, 512, 0), seg(1, 12, 1536, 512, 0)])
    add('B', 2, 1024, 1024, [seg(0, 3, 384, 1024, 0), seg(1, 4, 512, 1024, 0)])
    add('S', 2, 512, 384, [seg(0, 1, 1664, 384, 0), seg(1, 5, 1664, 384, 0)])
    add('B', 2, 1024, 1024, [seg(0, 5, 640, 1024, 0), seg(1, 6, 768, 1024, 0)])
    add('S', 2, 512, 384, [seg(0, 9, 1664, 384, 0), seg(1, 13, 1664, 384, 0)])
    add('B', 2, 1024, 1024, [seg(0, 7, 896, 1024, 0), seg(1, 8, 1024, 1024, 0)])
    add('S', 4, 256, 256, [seg(0, 2, 1792, 256, 0), seg(1, 6, 1792, 256, 0),
                           seg(2, 10, 1792, 256, 0), seg(3, 14, 1792, 256, 0)])
    # quad of 512-segments in one B tile (two per row)
    add('B', 2, 1024, 1024, [seg(0, 1, 1152, 512, 0), seg(0, 2, 1280, 512, 512),
                             seg(1, 3, 1408, 512, 0), seg(1, 4, 1536, 512, 512)])
    add('S', 4, 128, 128, [seg(0, 3, 1920, 128, 0), seg(1, 7, 1920, 128, 0),
                           seg(2, 11, 1920, 128, 0), seg(3, 15, 1920, 128, 0)])

    # sanity: every (jt, i-range) of the causal triangle covered exactly once
    cov = {}
    for t in tiles:
        for sg in t['segs']:
            for i in range(sg['i0'], sg['i0'] + sg['w'], 128):
                key = (sg['jt'], i // 128)
                assert key not in cov, key
                cov[key] = True
    assert len(cov) == sum(NJT - jt for jt in range(NJT))
    assert sum(sg['w'] for t in tiles for sg in t['segs']) == 17408
    return tiles


def _emit(tc: tile.TileContext, kq_d, v_d, o_d):
    nc = tc.nc
    Exp = mybir.ActivationFunctionType.Exp
    tiles = _unit_tiles()

    with (
        tc.tile_pool(name="singles", bufs=1) as singles,
        tc.tile_pool(name="wtB", bufs=WT_BUFS) as wtBpool,
        tc.tile_pool(name="wtS", bufs=WT_BUFS) as wtSpool,
        tc.tile_pool(name="out", bufs=O_BUFS) as opool,
        tc.tile_pool(name="qkB", bufs=1, space="PSUM") as qkBpool,
        tc.tile_pool(name="qkS", bufs=1, space="PSUM") as qkSpool,
        tc.tile_pool(name="avp", bufs=1, space="PSUM") as avpool,
    ):
        # trigger the ACT exp table load immediately so it overlaps the
        # input DMAs instead of stalling the first real exp (~1.3us)
        warm = singles.tile([1, 1], F32)
        nc.vector.memset(warm, 0.0)
        nc.scalar.activation(warm, warm, Exp, scale=1.0)

        # row 0 of dim 1: k, row 1: q (same SBUF tile -> one DMA per unit)
        kq_sb = singles.tile([CK, 2, UPC, S], F32R)
        v_sb = singles.tile([128, UPC, NJT, AVW], F32)
        nc.sync.dma_start(out=kq_sb[:, :, 0], in_=kq_d.ap()[:, :, 0])
        nc.sync.dma_start(out=v_sb[:, 0], in_=v_d.ap()[:, 0])
        for u in range(1, UPC):
            nc.sync.dma_start(out=kq_sb[:, :, u], in_=kq_d.ap()[:, :, u])
            nc.sync.dma_start(out=v_sb[:, u], in_=v_d.ap()[:, u])

        # trimask[p, f] = 1.0 if f >= p else 0.0 (keep j <= i on diag blocks)
        trimask = singles.tile([128, 128], F32)
        make_upper_triangular(nc, trimask[:], val=1.0, diag=True)

        # av ring: unit u accumulates into av_all[:, u % 2]; col 0 = denom.
        # padded so each ring slot owns a full PSUM bank: matmul start=True
        # clears has_written for the WHOLE bank, so the two in-flight units'
        # accumulators must not share one (and within a unit only the first
        # AV matmul may use start=True).
        av_all = avpool.tile([128, 2, NJT, AVW], F32,
                             padded_shape=[128, 2, NJT, 32])

        if PE_WARMUP:
            # dummy matmuls during the input DMA wait release the HAM clock
            # throttle (1.2 -> 2.4 GHz) before the first real QK matmul
            wsrc = singles.tile([CK, 512], F32R)
            nc.vector.memset(wsrc.bitcast(F32), 0.0)
            wp = qkBpool.tile([128, 2, 1024], F32, tag='B')
            for i in range(PE_WARMUP):
                nc.tensor.matmul(
                    wp[:, i % 2, 0:512],
                    lhsT=wsrc[:, 0:128],
                    rhs=wsrc,
                    start=True,
                    stop=True,
                )

        # global tile stream (3 units x 12 tiles), with first/last AV
        # contributor flags per (unit, i-tile)
        stream = []
        for u in range(UPC):
            for t in tiles:
                stream.append((u, t))
        contrib = {}
        for g, (u, t) in enumerate(stream):
            for sg in t['segs']:
                for it in range(sg['i0'] // 128, (sg['i0'] + sg['w']) // 128):
                    contrib.setdefault((u, it), []).append(g)

        def emit_qk(g):
            u, t = stream[g]
            k_sb = kq_sb[:, 0]
            q_sb = kq_sb[:, 1]
            if t['kind'] == 'B':
                qk = qkBpool.tile([128, 2, 1024], F32, tag='B')
            else:
                # pad 384-wide rows to a 512 stride so no row crosses a bank
                pad = [128, t['nrows'], 512] if t['rowlen'] == 384 else None
                qk = qkSpool.tile([128, t['nrows'], t['rowlen']], F32, tag='S',
                                  padded_shape=pad)
            for sg in t['segs']:
                for c in range(0, sg['w'], 512):
                    cw = min(512, sg['w'] - c)
                    nc.tensor.matmul(
                        qk[:, sg['r'], sg['c0'] + c:sg['c0'] + c + cw],
                        lhsT=k_sb[:, u, sg['jt'] * 128:(sg['jt'] + 1) * 128],
                        rhs=q_sb[:, u, sg['i0'] + c:sg['i0'] + c + cw],
                        start=True,
                        stop=True,
                    )
            return qk

        def emit_exp_mask(g, qk):
            u, t = stream[g]
            if t['kind'] == 'B':
                wt = wtBpool.tile([128, 2, 1024], F32, tag='B')
            else:
                wt = wtSpool.tile([128, t['nrows'], t['rowlen']], F32,
                                  tag='S')
            nc.scalar.activation(
                wt[:, :, 0:t['expw']], qk[:, :, 0:t['expw']], Exp, scale=SCALE
            )
            for sg in t['segs']:
                if sg['i0'] == sg['jt'] * 128:
                    # diagonal block: zero out j > i entries
                    d = slice(sg['c0'], sg['c0'] + 128)
                    nc.vector.tensor_mul(wt[:, sg['r'], d], wt[:, sg['r'], d],
                                         trimask)
            return wt

        n_av_per_unit = sum(NJT - jt for jt in range(NJT))
        av_idx = {}

        def emit_av(g, wt):
            u, t = stream[g]
            av = av_all[:, u % 2]
            for sg in t['segs']:
                for it in range(sg['i0'] // 128, (sg['i0'] + sg['w']) // 128):
                    o = sg['c0'] + it * 128 - sg['i0']
                    idx = av_idx.get(u, 0)
                    av_idx[u] = idx + 1
                    nc.tensor.matmul(
                        av[:, it, :],
                        lhsT=wt[:, sg['r'], o:o + 128],
                        rhs=v_sb[:, u, sg['jt'], :],
                        start=(idx == 0),
                        stop=(idx == n_av_per_unit - 1),
                        skip_group_check=True,
                    )

        def emit_normalize(u):
            av = av_all[:, u % 2]
            rcp = opool.tile([128, NJT], F32, tag='rcp')
            nc.vector.reciprocal_approx_fast(out=rcp, in_=av[:, :, 0])
            osb = opool.tile([128, NJT, CK], F32, tag='osb')
            rb = bass.AP(tensor=rcp.tensor, offset=rcp.offset,
                         ap=list(rcp.ap) + [[0, CK]])
            nc.vector.tensor_mul(osb, av[:, :, 1:1 + CK], rb)
            base = o_d.ap()
            ob = bass.AP(tensor=base.tensor,
                         offset=base.offset + u * NJT * 128 * CK,
                         ap=[[CK, 128], [128 * CK, NJT], [1, CK]])
            nc.sync.dma_start(out=ob, in_=osb)

        # PE program order: QK(T0), QK(T1), then AV(T_{g-2}), QK(T_g), ...
        # AV(T_k) and QK(T_{k+2}) both gate on exp(T_k) (same PSUM slot), so
        # this order adds no PE stalls while keeping the in-order PE queue
        # from blocking QK behind not-yet-ready AV work.
        n = len(stream)
        live = {}
        live[0] = emit_qk(0)
        live[1] = emit_qk(1)
        wts = {}
        for g in range(n):
            wts[g] = emit_exp_mask(g, live.pop(g))
            if g + 2 < n:
                emit_av(g, wts.pop(g))
                live[g + 2] = emit_qk(g + 2)
                # end of unit: normalize right after its last tile's AV
                u, t = stream[g]
                if g % len(tiles) == len(tiles) - 1:
                    pass
            else:
                emit_av(g, wts.pop(g))
            if g % len(tiles) == len(tiles) - 1:
                emit_normalize(g // len(tiles))


_PROGRAM = None


def _get_program():
    global _PROGRAM
    if _PROGRAM is None:
        nc = bacc.Bacc(
            "TRN2",
            target_bir_lowering=False,
            debug=False,
            num_devices=N_CORES,
        )
        kq_d = nc.declare_dram_parameter("kq", [CK, 2, UPC, S], F32R,
                                         isOutput=False)
        v_d = nc.declare_dram_parameter(
            "vaug", [128, UPC, NJT, AVW], F32, isOutput=False
        )
        # output laid out [unit, i-tile, i-within-tile, channel]
        o_d = nc.declare_dram_parameter("o", [UPC, NJT, 128, CK], F32,
                                        isOutput=True)
        with tile.TileContext(nc) as tc:
            _emit(tc, kq_d, v_d, o_d)
        if not nc.is_finalized():
            nc.finalize()
        _PROGRAM = nc
    return _PROGRAM


# test.py can flip this on to capture an NTFF trace / exec time.
TRACE = False
LAST_RESULTS = None


def kernel(keys, queries, values, attn_mask, num_heads):
    global LAST_RESULTS
    nh = int(num_heads)
    assert nh == NH, f"compiled for num_heads={NH}, got {nh}"
    assert keys.shape == (STACK, B, C, D, H, W)

    # (stack*b, head, ck, seq)
    q = np.ascontiguousarray(queries, np.float32).reshape(STACK * B, NH, CK, S)
    k = np.ascontiguousarray(keys, np.float32).reshape(STACK * B, NH, CK, S)
    v = np.ascontiguousarray(values, np.float32).reshape(STACK * B, NH, CK, S)

    in_maps = []
    for core in range(N_CORES):
        units = range(core * UPC, (core + 1) * UPC)
        qs = np.stack([q[u // NH, u % NH] for u in units], 1)  # [CK, UPC, S]
        ks = np.stack([k[u // NH, u % NH] for u in units], 1)
        vt = np.stack([v[u // NH, u % NH] for u in units], 0)  # [UPC, CK, S]
        kq = np.ascontiguousarray(np.stack([ks, qs], 1))       # [CK,2,UPC,S]
        vaug = np.zeros((128, UPC, NJT, AVW), np.float32)
        vaug[:, :, :, 0] = 1.0
        vaug[:, :, :, 1:] = vt.reshape(UPC, CK, NJT, 128).transpose(3, 0, 2, 1)
        in_maps.append({"kq": kq, "vaug": vaug})

    nc = _get_program()
    kwargs = {}
    if TRACE:
        kwargs = dict(trace=True, trace_cores=[0])
    LAST_RESULTS = run_bass_kernel_spmd(
        nc, in_maps, core_ids=list(range(N_CORES)), **kwargs
    )

    out = np.empty((STACK * B, NH, CK, S), np.float32)
    for core in range(N_CORES):
        o = LAST_RESULTS.results[core]["o"]  # [UPC, NJT, 128, CK]
        for j, u in enumerate(range(core * UPC, (core + 1) * UPC)):
            out[u // NH, u % NH] = o[j].reshape(S, CK).T
    return out.reshape(STACK, B, C, D, H, W)
